# revision 7
# baseline (speedup 1.0000x reference)
"""Trainium2 Bass kernel for nn_NodeNet (gnn_message_passing).

Math (after dead-code elimination of the reference):
  out = p1(angles) where angles = (Ri * e) @ (Ro.T @ X[:, :8])  [N, 8]
  p1(a) = || W1 @ (prod-state of a) ||^2 with W1 = rows of the fixed
  256x256 circuit matrix W(theta) where wire-5 bit == 1 (probability of
  measuring |1> on wire 5; equals (1-<Z5>)/2 since evolution is orthogonal).

Distribution (8 NeuronCores):
  L1 (node-sharded): core k computes H_k = X8_k.T @ Ro_k  [8, E] (partial).
  host: B = sum_k H_k;  Bw = e * B.T  [E, 8];  (also builds W1 from theta)
  L2 (node-sharded): core k computes miT = Bw.T-contract against Ri_k.T
     (streamed as host-pretransposed RiT_k), then runs the circuit on
     device (sincos w/ Cody-Waite reduction, product-state build,
     W1 contraction, |.|^2 accumulation) -> out_k [1024].
"""

import numpy as np
from contextlib import ExitStack

import concourse.bass as bass
import concourse.mybir as mybir
from concourse.bass_utils import run_bass_kernel_spmd

AF = mybir.ActivationFunctionType
ALU = mybir.AluOpType
F32 = mybir.dt.float32

NCORES = 8
N_NODES = 8192
E_EDGES = 16384
NSH = N_NODES // NCORES      # 1024 nodes per core
NW = 8                       # wires / used angle columns
NTILES = NSH // 128          # 8 node tiles of 128 per core
NGRP = 2                     # node groups per core in L2 (for overlap)
GSZ = NSH // NGRP            # 512 nodes per group
GT = GSZ // 128              # 4 tiles per group

# ---------------- sincos constants (f32 Cody-Waite for 2*pi) ----------------


def _mask_lo(x32, nbits):
    u = np.frombuffer(np.float32(x32).tobytes(), np.uint32)[0]
    u &= np.uint32(0xFFFFFFFF) << np.uint32(nbits)
    return np.frombuffer(np.uint32(u).tobytes(), np.float32)[0]


_TWO_PI64 = np.float64(2.0 * np.pi)
_C1 = _mask_lo(np.float32(_TWO_PI64), 15)                  # 9-bit mantissa
_C2 = _mask_lo(np.float32(_TWO_PI64 - np.float64(_C1)), 15)
_C3 = np.float32(_TWO_PI64 - np.float64(_C1) - np.float64(_C2))
_INV_4PI = np.float32(1.0 / (4.0 * np.pi))
_MAGIC = np.float32(1.5 * 2.0**23)
_HALF_PI = np.float32(np.pi / 2.0)

# ---------------- launch 1: H_k = X8_k.T @ Ro_k ----------------

_cache = {}


def _build_l1():
    EG = 2048                 # edge columns per psum round
    NEG = E_EDGES // EG       # 8 rounds
    NCH = NSH // 128          # 8 node chunks
    NBUF = 4

    nc = bass.Bass()
    ro = nc.declare_dram_parameter("ro", [NSH, E_EDGES], F32, isOutput=False)
    x8 = nc.declare_dram_parameter("x8", [128, NCH, NW], F32, isOutput=False)
    h = nc.declare_dram_parameter("h", [NW, E_EDGES], F32, isOutput=True)

    ro_t = ro.rearrange("(c p) e -> p c e", p=128)

    with ExitStack() as ctx:
        x8t = ctx.enter_context(nc.sbuf_tensor([128, NCH, NW], F32))
        rob = ctx.enter_context(nc.sbuf_tensor([128, NBUF, EG], F32))
        hsb = ctx.enter_context(nc.sbuf_tensor([NW, E_EDGES], F32))
        hps = ctx.enter_context(nc.psum_tensor([NW, 2 * EG], F32))
        dma_in = ctx.enter_context(nc.semaphore("dma_in"))
        dma_out = ctx.enter_context(nc.semaphore("dma_out"))
        pe_chunk = ctx.enter_context(nc.semaphore("pe_chunk"))
        act_cp = ctx.enter_context(nc.semaphore("act_cp"))
        block = ctx.enter_context(nc.Block())

        @block.sync
        def _(sync):
            sync.dma_start(x8t[:], x8[:]).then_inc(dma_in, 16)
            for eg in range(NEG):
                for c in range(NCH):
                    i = eg * NCH + c
                    if i >= NBUF:
                        sync.wait_ge(pe_chunk, i - NBUF + 1)
                    sync.dma_start(
                        rob[:, i % NBUF, :],
                        ro_t[:, c, eg * EG:(eg + 1) * EG],
                    ).then_inc(dma_in, 16)
            sync.wait_ge(act_cp, NEG)
            sync.dma_start(h[:], hsb[:]).then_inc(dma_out, 16)
            sync.wait_ge(dma_out, 16)

        @block.tensor
        def _(tensor):
            tensor.wait_ge(dma_in, 16)  # x8 loaded
            for eg in range(NEG):
                half = eg % 2
                for c in range(NCH):
                    i = eg * NCH + c
                    tensor.wait_ge(dma_in, 16 * (i + 2))
                    if c == 0 and eg >= 2:
                        tensor.wait_ge(act_cp, eg - 1)
                    for ns in range(EG // 512):
                        ins = nc.tensor.matmul(
                            hps[:, half * EG + ns * 512:half * EG + (ns + 1) * 512],
                            x8t[:, c, :],
                            rob[:, i % NBUF, ns * 512:(ns + 1) * 512],
                            start=(c == 0),
                            stop=(c == NCH - 1),
                        )
                    ins.then_inc(pe_chunk, 1)

        @block.scalar
        def _(scalar):
            for eg in range(NEG):
                half = eg % 2
                scalar.wait_ge(pe_chunk, NCH * (eg + 1))
                scalar.activation(
                    hsb[:, eg * EG:(eg + 1) * EG],
                    hps[:, half * EG:(half + 1) * EG],
                    AF.Copy,
                ).then_inc(act_cp, 1)

    return nc


# ---------------- launch 2: message matmul + circuit ----------------


def _build_l2(debug=False):
    NBUF = 3                  # rit stream buffers
    BE = 4                    # edge chunks (of 128) per DMA batch
    NBATCH = E_EDGES // (128 * BE)   # 32 batches per group

    nc = bass.Bass()
    rit = nc.declare_dram_parameter("rit", [E_EDGES, NSH], F32, isOutput=False)
    bw = nc.declare_dram_parameter("bw", [128, E_EDGES // 128, NW], F32, isOutput=False)
    w1t = nc.declare_dram_parameter("w1t", [2, 128, 128], F32, isOutput=False)
    idm = nc.declare_dram_parameter("idm", [128, 128], F32, isOutput=False)
    cst = nc.declare_dram_parameter("cst", [128, 1], F32, isOutput=False)
    yout = nc.declare_dram_parameter("y", [NTILES, 128], F32, isOutput=True)
    if debug:
        dbg_mi = nc.declare_dram_parameter("dbg_mi", [NW, NGRP, GSZ], F32, isOutput=True)
        dbg_ang = nc.declare_dram_parameter("dbg_ang", [128, NGRP, GT * NW], F32, isOutput=True)
        dbg_r = nc.declare_dram_parameter("dbg_r", [128, NGRP, GT * NW], F32, isOutput=True)
        dbg_cs = nc.declare_dram_parameter("dbg_cs", [128, NGRP, 2, GT * NW], F32, isOutput=True)
        dbg_st = nc.declare_dram_parameter("dbg_st", [128, 2, 256], F32, isOutput=True)
        dbg_pout = nc.declare_dram_parameter("dbg_pout", [128, NW], F32, isOutput=True)

    rit_t = rit.rearrange("(b p) n -> p b n", p=128)
    NWD = 5  # number of initial weight dmas

    with ExitStack() as ctx:
        ec_ = ctx.enter_context
        bwt = ec_(nc.sbuf_tensor([128, E_EDGES // 128, NW], F32))
        w1sb = ec_(nc.sbuf_tensor([128, 2, 128], F32))
        idt = ec_(nc.sbuf_tensor([128, 128], F32))
        halfpi = ec_(nc.sbuf_tensor([128, 1], F32))
        ritb = ec_(nc.sbuf_tensor([128, NBUF, BE, GSZ], F32))
        misb = ec_(nc.sbuf_tensor([NW, NGRP, GSZ], F32))
        micomb = ec_(nc.sbuf_tensor([NW, 4, GSZ], F32))
        ang = ec_(nc.sbuf_tensor([128, NGRP, GT * NW], F32))
        xh = ec_(nc.sbuf_tensor([128, NGRP, GT * NW], F32))
        kv = ec_(nc.sbuf_tensor([128, NGRP, GT * NW], F32))
        tv = ec_(nc.sbuf_tensor([128, NGRP, GT * NW], F32))
        rv = ec_(nc.sbuf_tensor([128, NGRP, GT * NW], F32))
        av = ec_(nc.sbuf_tensor([128, NGRP, GT * NW], F32))
        avs = ec_(nc.sbuf_tensor([128, NGRP, GT * NW], F32))
        cs = ec_(nc.sbuf_tensor([128, NGRP, 2, GT * NW], F32))
        stA = ec_(nc.sbuf_tensor([128, 128], F32))
        stB = ec_(nc.sbuf_tensor([128, 2, 256], F32))
        sTt = ec_(nc.sbuf_tensor([128, 2, 2, 128], F32))
        scrq = ec_(nc.sbuf_tensor([128, 128], F32))
        pout = ec_(nc.sbuf_tensor([128, NW], F32))
        ysb = ec_(nc.sbuf_tensor([NTILES, 128], F32))
        mips = ec_(nc.psum_tensor([NW, 4 * 512], F32))      # 4 banks (interleaved)
        angp = ec_(nc.psum_tensor([128, 512], F32))         # 1 bank
        tps0 = ec_(nc.psum_tensor([128, 512], F32))         # 1 bank (transposes + y2)
        tps1 = ec_(nc.psum_tensor([128, 512], F32))         # 1 bank (transposes + y2)
        pps = ec_(nc.psum_tensor([NW, 512], F32))           # 1 bank
        dma_in = ec_(nc.semaphore("dma_in"))
        dma_out = ec_(nc.semaphore("dma_out"))
        pe_mi = ec_(nc.semaphore("pe_mi"))
        pe_ang = ec_(nc.semaphore("pe_ang"))
        pe_tr = ec_(nc.semaphore("pe_tr"))
        pe_y2 = ec_(nc.semaphore("pe_y2"))
        pe_fin = ec_(nc.semaphore("pe_fin"))
        v_mi = ec_(nc.semaphore("v_mi"))
        v_micp = ec_(nc.semaphore("v_micp"))
        v_ang = ec_(nc.semaphore("v_ang"))
        v_sc = ec_(nc.semaphore("v_sc"))
        v_build = ec_(nc.semaphore("v_build"))
        v_st = ec_(nc.semaphore("v_st"))
        v_out = ec_(nc.semaphore("v_out"))
        a_sc = ec_(nc.semaphore("a_sc"))
        a_sq = ec_(nc.semaphore("a_sq"))
        block = ec_(nc.Block())

        tps = [tps0, tps1]

        @block.sync
        def _(sync):
            sync.dma_start(bwt[:], bw[:]).then_inc(dma_in, 16)
            sync.dma_start(w1sb[:, 0, :], w1t[0]).then_inc(dma_in, 16)
            sync.dma_start(w1sb[:, 1, :], w1t[1]).then_inc(dma_in, 16)
            sync.dma_start(idt[:], idm[:]).then_inc(dma_in, 16)
            sync.dma_start(halfpi[:], cst[:]).then_inc(dma_in, 16)
            for g in range(NGRP):
                for b in range(NBATCH):
                    i = g * NBATCH + b
                    if i >= NBUF:
                        sync.wait_ge(pe_mi, i - NBUF + 1)
                    sync.dma_start(
                        ritb[:, i % NBUF, :, :],
                        rit_t[:, b * BE:(b + 1) * BE, g * GSZ:(g + 1) * GSZ],
                    ).then_inc(dma_in, 16)
            sync.wait_ge(v_out, 1)
            sync.dma_start(yout[:], ysb[:]).then_inc(dma_out, 16)
            nout = 1
            if debug:
                for src_ap, dst in [(misb[:], dbg_mi), (ang[:], dbg_ang),
                                    (rv[:], dbg_r), (cs[:], dbg_cs),
                                    (stB[:], dbg_st), (pout[:], dbg_pout)]:
                    sync.dma_start(dst[:], src_ap).then_inc(dma_out, 16)
                    nout += 1
            sync.wait_ge(dma_out, 16 * nout)

        @block.tensor
        def _(tensor):
            def accum_group(g):
                if g > 0:
                    tensor.wait_ge(v_micp, g)  # 4-bank combine of prev group done
                for b in range(NBATCH):
                    i = g * NBATCH + b
                    tensor.wait_ge(dma_in, 16 * (NWD + i + 1))
                    for j in range(BE):
                        ins = nc.tensor.matmul(
                            mips[:, j * 512:j * 512 + GSZ],
                            bwt[:, b * BE + j, :],
                            ritb[:, i % NBUF, j, :],
                            start=(b == 0),
                            stop=(b == NBATCH - 1),
                        )
                    ins.then_inc(pe_mi, 1)

            def angle_transposes(g):
                # miT [8, GSZ] -> angles tiles [128, 8] each, into angp psum
                tensor.wait_ge(v_mi, g + 1)
                if g > 0:
                    tensor.wait_ge(v_ang, g)  # V done reading angp for g-1
                for t in range(GT):
                    ins = nc.tensor.transpose(
                        angp[:, t * NW:(t + 1) * NW],
                        misb[:, g, t * 128:(t + 1) * 128],
                        idt[:NW, :NW],
                    )
                ins.then_inc(pe_ang, 1)

            def circuit_tiles(g):
                for t in range(GT):
                    jt = g * GT + t
                    par = jt % 2
                    # state transposes: stB [128, 256] -> sT 2x[128, 128]
                    tensor.wait_ge(v_build, jt + 1)
                    if jt >= 2:
                        tensor.wait_ge(v_st, jt - 1)  # V copied tps[par] of jt-2
                        tensor.wait_ge(a_sq, jt - 1)  # ACT done reading this bank
                    for hh in range(2):
                        ins = nc.tensor.transpose(
                            tps[par][:, hh * 128:(hh + 1) * 128],
                            stB[:, par, hh * 128:(hh + 1) * 128],
                            idt[:],
                        )
                    ins.then_inc(pe_tr, 1)
                    # y2 = S @ W1.T : [128 nodes, 128 rows] into same bank cols 256:384
                    tensor.wait_ge(v_st, jt + 1)
                    for hh in range(2):
                        ins = nc.tensor.matmul(
                            tps[par][:, 256:384],
                            sTt[:, par, hh, :],
                            w1sb[:, hh, :],
                            start=(hh == 0),
                            stop=(hh == 1),
                        )
                    ins.then_inc(pe_y2, 1)

            accum_group(0)
            angle_transposes(0)
            accum_group(1)
            circuit_tiles(0)
            angle_transposes(1)
            circuit_tiles(1)
            # final output transpose pout [128, 8] -> [8, 128]
            tensor.wait_ge(a_sq, NGRP * GT)
            nc.tensor.transpose(pps[:, :128], pout[:], idt[:]).then_inc(pe_fin, 1)

        @block.vector
        def _(vector):
            for g in range(NGRP):
                # 4-bank interleaved miT combine: psum banks -> sbuf, pairwise add
                vector.wait_ge(pe_mi, NBATCH * (g + 1))
                for j in range(4):
                    vector.tensor_copy(micomb[:, j, :], mips[:, j * 512:j * 512 + GSZ])
                vector.drain()
                vector.nop().then_inc(v_micp, 1)
                vector.tensor_tensor(
                    micomb[:, 0, :], micomb[:, 0, :], micomb[:, 1, :], ALU.add
                )
                vector.tensor_tensor(
                    micomb[:, 2, :], micomb[:, 2, :], micomb[:, 3, :], ALU.add
                )
                vector.drain()
                vector.tensor_tensor(
                    misb[:, g, :], micomb[:, 0, :], micomb[:, 2, :], ALU.add
                )
                vector.drain()
                vector.nop().then_inc(v_mi, 1)
                # angles psum -> sbuf
                vector.wait_ge(pe_ang, g + 1)
                vector.tensor_copy(ang[:, g, :], angp[:, :GT * NW]).then_inc(v_ang, 1)
                # sincos range reduction: r = (x/2) mod 2pi -> [-pi, pi]
                # NOTE: raw-bass DVE ops have no intra-engine RAW protection;
                # drain() between dependent ops is mandatory.
                vector.drain()
                a = ang[:, g, :]
                vector.tensor_scalar_mul(xh[:, g, :], a, float(np.float32(0.5)))
                vector.tensor_scalar_mul(kv[:, g, :], a, float(_INV_4PI))
                vector.drain()
                vector.tensor_scalar(
                    kv[:, g, :], kv[:, g, :], float(_MAGIC), float(-_MAGIC),
                    ALU.add, ALU.add,
                )
                vector.drain()
                vector.tensor_scalar_mul(tv[:, g, :], kv[:, g, :], float(_C1))
                vector.tensor_scalar_mul(av[:, g, :], kv[:, g, :], float(_C2))
                vector.drain()
                vector.tensor_tensor(rv[:, g, :], xh[:, g, :], tv[:, g, :], ALU.subtract)
                vector.tensor_scalar_mul(tv[:, g, :], kv[:, g, :], float(_C3))
                vector.drain()
                vector.tensor_tensor(rv[:, g, :], rv[:, g, :], av[:, g, :], ALU.subtract)
                vector.drain()
                ins = vector.tensor_tensor(
                    rv[:, g, :], rv[:, g, :], tv[:, g, :], ALU.subtract
                )
                vector.drain()
                ins.then_inc(v_sc, 1)
                # product state build per tile
                vector.wait_ge(a_sc, g + 1)
                for t in range(GT):
                    jt = g * GT + t
                    par = jt % 2
                    if jt >= 2:
                        vector.wait_ge(pe_tr, jt - 1)  # stB[par] free (transposed)
                    col = t * NW
                    c_ap = cs[:, g, 1, :]
                    s_ap = cs[:, g, 0, :]
                    # w = 0 into stA[:, 0:2]
                    vector.tensor_copy(stA[:, 0:1], c_ap[:, col:col + 1])
                    vector.tensor_copy(stA[:, 1:2], s_ap[:, col:col + 1])
                    cur = stA[:, 0:2]
                    for w in range(1, NW):
                        vector.drain()
                        L = 1 << w
                        if w == NW - 1:
                            nxt = stB[:, par, :]
                        elif w % 2 == 1:
                            nxt = stB[:, par, :2 * L]
                        else:
                            nxt = stA[:, :2 * L]
                        n3 = nxt.rearrange("p (l two) -> p l two", two=2)
                        vector.tensor_scalar_mul(n3[:, :, 0], cur, c_ap[:, col + w:col + w + 1])
                        ins = vector.tensor_scalar_mul(
                            n3[:, :, 1], cur, s_ap[:, col + w:col + w + 1]
                        )
                        cur = nxt
                    vector.drain()
                    ins.then_inc(v_build, 1)
                    # copy tps -> sTt when PE transposes done
                    vector.wait_ge(pe_tr, jt + 1)
                    if jt >= 2:
                        vector.wait_ge(pe_y2, jt - 1)  # sTt[par] free
                    vector.tensor_copy(sTt[:, par, 0, :], tps[par][:, 0:128])
                    vector.tensor_copy(sTt[:, par, 1, :], tps[par][:, 128:256]).then_inc(
                        v_st, 1
                    )
            # final output copy
            vector.wait_ge(pe_fin, 1)
            vector.tensor_copy(ysb[:], pps[:, :128]).then_inc(v_out, 1)

        @block.scalar
        def _(scalar):
            scalar.wait_ge(dma_in, 16 * NWD)
            for g in range(NGRP):
                scalar.wait_ge(v_sc, g + 1)
                # s = sin(r); c = sin(pi/2 - |r|) = cos(r)
                scalar.activation(cs[:, g, 0, :], rv[:, g, :], AF.Sin)
                scalar.activation(avs[:, g, :], rv[:, g, :], AF.Abs)
                scalar.drain()
                scalar.activation(
                    cs[:, g, 1, :], avs[:, g, :], AF.Sin, bias=halfpi[:], scale=-1.0
                ).then_inc(a_sc, 1)
                for t in range(GT):
                    jt = g * GT + t
                    par = jt % 2
                    scalar.wait_ge(pe_y2, jt + 1)
                    scalar.activation(
                        scrq[:], tps[par][:, 256:384], AF.Square,
                        accum_out=pout[:, jt:jt + 1],
                    ).then_inc(a_sq, 1)

    return nc


# ---------------- host: circuit matrix W(theta) ----------------


def _build_W1T(theta):
    NWIRES = 8
    dt = np.float64
    CNOT2 = np.array([[1, 0, 0, 0], [0, 1, 0, 0], [0, 0, 0, 1], [0, 0, 1, 0]], dtype=dt)

    eyes = [np.eye(1 << k, dtype=dt) for k in range(9)]

    def ry_mat(th, w):
        c, s = np.cos(dt(th) / 2), np.sin(dt(th) / 2)
        U2 = np.array([[c, -s], [s, c]], dtype=dt)
        return np.kron(eyes[w], np.kron(U2, eyes[NWIRES - 1 - w]))

    def cnot_mat(cw, tw):
        M = np.zeros((256, 256), dtype=dt)
        for idx in range(256):
            bits = [(idx >> (7 - i)) & 1 for i in range(8)]
            if bits[cw] == 1:
                bits[tw] ^= 1
            j = sum(b << (7 - i) for i, b in enumerate(bits))
            M[j, idx] = 1.0
        return M

    seq = [("ry", 0, 0), ("ry", 1, 1), ("cn", 0, 1), ("ry", 2, 2), ("ry", 3, 3),
           ("cn", 3, 2), ("ry", 4, 4), ("ry", 5, 5), ("cn", 4, 5), ("ry", 6, 6),
           ("ry", 7, 7), ("cn", 7, 6), ("ry", 8, 1), ("ry", 9, 2), ("cn", 1, 2),
           ("ry", 10, 5), ("ry", 11, 6), ("cn", 6, 5), ("ry", 12, 2), ("ry", 13, 5),
           ("cn", 2, 5), ("ry", 14, 5)]
    W = np.eye(256, dtype=dt)
    for kind, a, b in seq:
        G = ry_mat(theta[a], b) if kind == "ry" else cnot_mat(a, b)
        W = G @ W
    rows1 = [r for r in range(256) if (r >> 2) & 1 == 1]
    W1 = W[rows1, :]                       # [128, 256]
    return np.ascontiguousarray(W1.T.astype(np.float32))   # [256, 128]


def _fast_transpose(a):
    """Contiguous transpose of a 2D float32 array, blocked for cache."""
    rows, cols = a.shape
    out = np.empty((cols, rows), np.float32)
    bs = 256
    for r0 in range(0, rows, bs):
        r1 = min(r0 + bs, rows)
        out[:, r0:r1] = a[r0:r1, :].T
    return out


# ---------------- public entry ----------------


def kernel(X, e, Ri, Ro, theta_learn):
    X = np.asarray(X, dtype=np.float32)
    e = np.asarray(e, dtype=np.float32)
    Ri = np.asarray(Ri, dtype=np.float32)
    Ro = np.asarray(Ro, dtype=np.float32)
    theta = np.asarray(theta_learn, dtype=np.float64)

    core_ids = list(range(NCORES))

    if "l1" not in _cache:
        _cache["l1"] = _build_l1()
    if "l2" not in _cache:
        _cache["l2"] = _build_l2()

    X8 = np.ascontiguousarray(X[:, :NW])

    # ---- launch 1 ----
    in1 = []
    for k in range(NCORES):
        x8k = X8[k * NSH:(k + 1) * NSH]
        x8s = np.ascontiguousarray(x8k.reshape(NSH // 128, 128, NW).transpose(1, 0, 2))
        in1.append({"ro": Ro[k * NSH:(k + 1) * NSH], "x8": x8s})
    res1 = run_bass_kernel_spmd(_cache["l1"], in1, core_ids, **_trace_kw())
    _record_time("l1", res1)
    H = res1.results[0]["h"].astype(np.float32)
    for k in range(1, NCORES):
        H = H + res1.results[k]["h"]
    Bw = e[:, None] * H.T                                   # [E, 8] f32
    bws = np.ascontiguousarray(Bw.reshape(E_EDGES // 128, 128, NW).transpose(1, 0, 2))

    # ---- host-side circuit matrix + consts ----
    w1t = _build_W1T(theta).reshape(2, 128, 128)
    idm = np.eye(128, dtype=np.float32)
    cst = np.full((128, 1), _HALF_PI, np.float32)

    # ---- launch 2 ----
    in2 = []
    for k in range(NCORES):
        ritk = _fast_transpose(Ri[k * NSH:(k + 1) * NSH])   # [E, NSH]
        in2.append({"rit": ritk, "bw": bws, "w1t": w1t, "idm": idm, "cst": cst})
    res2 = run_bass_kernel_spmd(_cache["l2"], in2, core_ids, **_trace_kw())
    _record_time("l2", res2)

    out = np.empty(N_NODES, np.float32)
    for k in range(NCORES):
        out[k * NSH:(k + 1) * NSH] = res2.results[k]["y"].reshape(NSH)
    return out


# ---------------- optional tracing (used by test.py) ----------------

_last_exec_times = {}


def _trace_kw():
    import os

    if os.environ.get("NODENET_TRACE"):
        return dict(trace=True)
    return {}


def _record_time(name, res):
    _last_exec_times[name] = {
        "exec_time_ns": res.exec_time_ns,
        "mean_exec_time_ns": res.mean_exec_time_ns,
    }
